# revision 13
# baseline (speedup 1.0000x reference)
"""Trainium2 Bass kernel for nn_DecoderCBatchNorm_63788854280467 (pipelined).

See kernel.py v2 notes. v3 adds a software pipeline over the 4 (batch, th)
units: stage F (front-end, per batch), stage A (gather + bilinear weighting),
stage B (MLP). Emission interleaves B(u) with A(u+1) so the latency-bound MLP
chain hides under the gather/weighting throughput work, and the MLP residual
adds run as PSUM identity-matmul accumulations (stage B is nearly DVE-free).
"""

import sys

sys.path.insert(0, "/opt/trn_rl_repo")

import numpy as np

import concourse.bass as bass
import concourse.bacc as bacc
import concourse.mybir as mybir
from concourse import tile, library_config
from concourse.bass_utils import run_bass_kernel_spmd
from concourse.masks import make_identity

F32 = mybir.dt.float32
F16 = mybir.dt.float16
I16 = mybir.dt.int16
AOT = mybir.AluOpType
AFT = mybir.ActivationFunctionType

B, T, L, H, W, D = 16, 4096, 4, 128, 128, 32
MAX_DIM = 0.55
NCORES = 8
BPC = B // NCORES          # batches per core = 2
MAGIC = 12582912.0         # 1.5 * 2^23 : f32 RNE rounding constant
RINTERVAL = 63.5           # (H-1)/2 exactly

# cst column map
_CN_PROJ = 0      # 48 cols: (b*24) + (2l+c)*3 + j
_CN_RDEN = 48     # 16 cols: b*8 + 2l+c : 1/den
_CN_B0 = 64       # 5 cols
_CN_CB = 69       # 6 cols: fc_p_b + cumsum(blocks_b1) (tiled 4x)
_CN_FOB = 75      # 1 col
_CN = 76

_cache = {}


def _ap3(tile_ap, dims, offset_elems):
    """Build an AP with explicit free dims [(step, count), ...] on a tile AP."""
    base = tile_ap
    ap = [list(base.ap[0])] + [[s, c] for (s, c) in dims]
    return bass.AP(base.tensor, base.offset + offset_elems, ap)


def _build_nc(iters=0, unroll=0):
    """Build the per-core program. iters>0 wraps the body in a timing loop;
    unroll>0 repeats the body inline (for TimelineSim steady-state)."""
    nc = bacc.Bacc("TRN2", target_bir_lowering=False, debug=False,
                   num_swdge_queues=4)

    ct = nc.dram_tensor("ct", [BPC * 2, 2 * H * W, 4 * D], F16, kind="ExternalInput")
    p_d = nc.dram_tensor("p", [BPC, T, 3], F32, kind="ExternalInput")
    pq_d = nc.dram_tensor("pq", [BPC, T, 3], F32, kind="ExternalInput")
    cst_d = nc.dram_tensor("cst", [128, _CN], F32, kind="ExternalInput")
    wbd_d = nc.dram_tensor("wbd", [128, 1280], F16, kind="ExternalInput")
    emat_d = nc.dram_tensor("emat", [128, 1024], F32, kind="ExternalInput")
    nmat_d = nc.dram_tensor("nmat", [96, 1024], F16, kind="ExternalInput")
    fob_d = nc.dram_tensor("fob", [128, 4], F16, kind="ExternalInput")
    o_d = nc.dram_tensor("o", [BPC, T], F32, kind="ExternalOutput")

    with tile.TileContext(nc) as tc:
        nc.gpsimd.load_library(library_config.mlp)
        with tc.tile_pool(name="sb", bufs=4) as pl, \
             tc.tile_pool(name="big", bufs=2) as big, \
             tc.tile_pool(name="cs", bufs=1) as cs, \
             tc.tile_pool(name="cfp", bufs=3) as cfp, \
             tc.tile_pool(name="ps", bufs=2, space="PSUM") as ps, \
             tc.tile_pool(name="pnb", bufs=2, space="PSUM") as pnb, \
             tc.tile_pool(name="pc", bufs=1, space="PSUM") as pc, \
             tc.tile_pool(name="pout", bufs=1, space="PSUM") as pout, \
             tc.tile_pool(name="ps1", bufs=1, space="PSUM") as ps1:

            ident = cs.tile([128, 128], F32)
            make_identity(nc, ident[:])
            ident16 = cs.tile([128, 128], F16)
            nc.vector.tensor_copy(out=ident16[:], in_=ident[:])
            cst = cs.tile([128, _CN], F32)
            nc.sync.dma_start(out=cst[:], in_=cst_d.ap())
            emat = cs.tile([128, 1024], F32)
            nc.sync.dma_start(out=emat[:], in_=emat_d.ap())
            wbd = cs.tile([128, 1280], F16)
            nc.sync.dma_start(out=wbd[:], in_=wbd_d.ap())
            nmat = cs.tile([96, 1024], F16)
            nc.sync.dma_start(out=nmat[:], in_=nmat_d.ap())
            fob = cs.tile([128, 4], F16)
            nc.sync.dma_start(out=fob[:], in_=fob_d.ap())

            def bias(col):
                return cst[:, col:col + 1]

            def stageF(b, st):
                """Front-end for batch b: Lt, DD, IDX (both th). Batch 0's
                chain runs on GPSIMD so the next iteration's gathers are not
                gated by the DVE weighting backlog; batch 1's runs on DVE."""
                eng = nc.gpsimd if b == 0 else nc.vector
                p_sb = pl.tile([128, 96], F32, tag="p", name="p_sb")
                nc.sync.dma_start(
                    out=p_sb[:],
                    in_=p_d.ap()[b].rearrange("(q a) j -> q (a j)", a=32))
                q_sb = pl.tile([128, 96], F32, tag="q", name="q_sb")
                nc.sync.dma_start(
                    out=q_sb[:],
                    in_=pq_d.ap()[b].rearrange("(q a) j -> q (a j)", a=32))

                pb1 = ps1.tile([128, 1024], F32, tag="pbank", name="ptp")
                nc.tensor.transpose(out=pb1[0:96, 0:128], in_=p_sb[:],
                                    identity=ident[:])
                pTs = pl.tile([128, 128], F16, tag="pts", name="pTs")
                nc.scalar.activation(out=pTs[0:96, :], in_=pb1[0:96, 0:128],
                                     func=AFT.Copy)
                st["pTs"] = pTs

                def qv(j):       # q_sb coord j, bcast over 8 (l,c)
                    return _ap3(q_sb[:], [(3, 32), (0, 8)], j)

                def cv(j):       # cst proj coeff per (l,c), bcast over a
                    return _ap3(cst[:], [(0, 32), (3, 8)], 24 * b + j)

                def nt(tag):
                    return pl.tile([128, 256], F32, tag=tag, name=tag)

                def uview(t):    # [128,256] as [(8,32a),(1,8lc)]
                    return _ap3(t[:], [(8, 32), (1, 8)], 0)

                T0 = nt("t0")
                eng.tensor_tensor(out=uview(T0), in0=qv(0), in1=cv(0),
                                        op=AOT.mult)
                T1 = nt("t1")
                eng.tensor_tensor(out=uview(T1), in0=qv(1), in1=cv(1),
                                        op=AOT.mult)
                T2 = nt("t2")
                eng.tensor_tensor(out=T2[:], in0=T0[:], in1=T1[:],
                                        op=AOT.add)
                T3 = nt("t0")
                eng.tensor_tensor(out=uview(T3), in0=qv(2), in1=cv(2),
                                        op=AOT.mult)
                U3 = nt("t1")
                eng.tensor_tensor(out=U3[:], in0=T2[:], in1=T3[:],
                                        op=AOT.add)
                Z = nt("t0")
                eng.tensor_tensor(
                    out=Z[:], in0=U3[:],
                    in1=_ap3(cst[:], [(0, 32), (1, 8)], _CN_RDEN + 8 * b),
                    op=AOT.mult)
                W1 = nt("t1")
                eng.tensor_scalar(out=W1[:], in0=Z[:], scalar1=1.0,
                                        scalar2=RINTERVAL, op0=AOT.add,
                                        op1=AOT.mult)
                XY2 = nt("xy2")
                eng.tensor_scalar(out=XY2[:], in0=W1[:], scalar1=126.9,
                                        scalar2=0.0, op0=AOT.min, op1=AOT.max)
                L1 = nt("t0")
                eng.tensor_scalar(out=L1[:], in0=XY2[:], scalar1=-0.5,
                                        scalar2=MAGIC, op0=AOT.add, op1=AOT.add)
                Lt = nt("lt")
                eng.tensor_scalar(out=Lt[:], in0=L1[:], scalar1=-MAGIC,
                                        scalar2=None, op0=AOT.add)
                # DD: cols [0,256) D1 (low-corner weight), [256,512) D2,
                # layout 128*coord + 4a + l within each half.
                DD = pl.tile([128, 512], F16, tag="dd", name="DD")
                eng.tensor_tensor(
                    out=_ap3(DD[:], [(128, 2), (4, 32), (1, 4)], 256),
                    in0=_ap3(XY2[:], [(1, 2), (8, 32), (2, 4)], 0),
                    in1=_ap3(Lt[:], [(1, 2), (8, 32), (2, 4)], 0),
                    op=AOT.subtract)
                eng.tensor_scalar(out=DD[:, 0:256], in0=DD[:, 256:512],
                                        scalar1=-1.0, scalar2=1.0,
                                        op0=AOT.mult, op1=AOT.add)
                st["DD"] = DD

                # Ft = 128*xl + yl  [128, 128] col 32l + a
                FA = pl.tile([128, 128], F32, tag="fa", name="FA")
                eng.tensor_scalar(
                    out=FA[:], in0=_ap3(Lt[:], [(2, 4), (8, 32)], 0),
                    scalar1=128.0, scalar2=None, op0=AOT.mult)
                Ft = pl.tile([128, 128], F32, tag="ft", name="Ft")
                eng.tensor_tensor(
                    out=Ft[:], in0=FA[:],
                    in1=_ap3(Lt[:], [(2, 4), (8, 32)], 1), op=AOT.add)
                eng.tensor_scalar(
                    out=_ap3(Ft[:], [(64, 2), (1, 32)], 32),
                    in0=_ap3(Ft[:], [(64, 2), (1, 32)], 32),
                    scalar1=16384.0, scalar2=None, op0=AOT.add)

                pidx = ps1.tile([128, 1024], F32, tag="pbank", name="pidx")
                for hh in range(8):
                    nc.tensor.matmul(out=pidx[:, 128 * hh:128 * hh + 128],
                                     lhsT=emat[:, 128 * hh:128 * hh + 128],
                                     rhs=Ft[:], start=True, stop=True)
                idxs = []
                for th in range(2):
                    IDX = pl.tile([128, 512], I16, tag="idx", name="IDX")
                    nc.scalar.activation(
                        out=_ap3(IDX[:], [(1, 8), (128, 4), (8, 16)], 0),
                        in_=_ap3(pidx[:], [(128, 8), (32, 4), (1, 16)], 16 * th),
                        func=AFT.Copy)
                    idxs.append(IDX)
                st["idx"] = idxs

            def stageA(b, th, st, stu):
                """Gather + bilinear weighting for unit (b, th) -> CFX, X0."""
                DD = st["DD"]
                Wt = pl.tile([128, 256], F16, tag="wt", name="Wt")
                for i in range(2):
                    nc.vector.tensor_tensor(
                        out=_ap3(Wt[:], [(16, 16), (4, 4), (1, 2)], 2 * i),
                        in0=_ap3(DD[:], [(4, 16), (1, 4), (0, 2)],
                                 256 * i + 64 * th),
                        in1=_ap3(DD[:], [(4, 16), (1, 4), (256, 2)],
                                 128 + 64 * th),
                        op=AOT.mult)
                G = big.tile([128, 8192], F16, tag="g", name="G")
                IDX = st["idx"][th]
                for lp in range(2):
                    nc.gpsimd.dma_gather(
                        out_ap=G[:, 4096 * lp:4096 * (lp + 1)]
                            .rearrange("q (j e) -> q j e", e=128),
                        in_ap=ct.ap()[b * 2 + lp],
                        idxs_ap=IDX[:, 256 * lp:256 * (lp + 1)],
                        num_idxs=4096, num_idxs_reg=4096,
                        elem_size=128, single_packet=False,
                        queue_num=(2 * (2 * b + th) + lp) % 4)
                yield
                GW = big.tile([128, 8192], F16, tag="gw", name="GW")
                for l in range(L):
                    nc.vector.tensor_tensor(
                        out=_ap3(GW[:], [(128, 16), (4, 32), (1, 4)], 2048 * l),
                        in0=_ap3(G[:], [(128, 16), (4, 32), (1, 4)], 2048 * l),
                        in1=_ap3(Wt[:], [(16, 16), (0, 32), (1, 4)], 4 * l),
                        op=AOT.mult)
                yield
                T01 = pl.tile([128, 2048], F16, tag="t01", name="T01")
                nc.vector.tensor_tensor(out=T01[:], in0=GW[:, 0:2048],
                                        in1=GW[:, 2048:4096], op=AOT.add)
                T23 = pl.tile([128, 2048], F16, tag="t23", name="T23")
                nc.vector.tensor_tensor(out=T23[:], in0=GW[:, 4096:6144],
                                        in1=GW[:, 6144:8192], op=AOT.add)
                TS = pl.tile([128, 2048], F16, tag="tsum", name="TS")
                nc.vector.tensor_tensor(out=TS[:], in0=T01[:], in1=T23[:],
                                        op=AOT.add)
                SH = pl.tile([128, 1024], F16, tag="sh", name="SH")
                nc.vector.tensor_tensor(
                    out=_ap3(SH[:], [(64, 16), (2, 32), (1, 2)], 0),
                    in0=_ap3(TS[:], [(128, 16), (4, 32), (1, 2)], 0),
                    in1=_ap3(TS[:], [(128, 16), (4, 32), (1, 2)], 2),
                    op=AOT.add)
                CFN = pl.tile([128, 512], F16, tag="cfn", name="CFN")
                nc.vector.tensor_tensor(
                    out=_ap3(CFN[:], [(32, 16), (1, 32)], 0),
                    in0=_ap3(SH[:], [(64, 16), (2, 32)], 0),
                    in1=_ap3(SH[:], [(64, 16), (2, 32)], 1),
                    op=AOT.add)
                yield
                CFX = cfp.tile([128, 512], F16, tag="cfx", name="CFX")
                NB = pnb.tile([128, 512], F32, tag="nb", name="NB")
                for k in range(4):
                    pcf = pc.tile([128, 128], F16, tag="pcf", name="pcf")
                    nc.tensor.transpose(out=pcf[:],
                                        in_=CFN[:, 128 * k:128 * k + 128],
                                        identity=ident16[:])
                    nc.scalar.activation(out=CFX[:, 128 * k:128 * (k + 1)],
                                         in_=pcf[:], func=AFT.Copy)
                    nc.tensor.matmul(
                        out=NB[:, 128 * k:128 * k + 128],
                        lhsT=nmat[:, 128 * (4 * th + k):128 * (4 * th + k) + 128],
                        rhs=st["pTs"][0:96, :], start=(k == 0), stop=False,
                        skip_group_check=True)
                for k in range(4):
                    nc.tensor.matmul(out=NB[:, 128 * k:128 * k + 128],
                                     lhsT=ident16[:],
                                     rhs=CFX[:, 128 * k:128 * k + 128],
                                     start=False, stop=False,
                                     skip_group_check=True)
                stu["CFX"] = CFX
                stu["NB"] = NB

            def emit_block(stu, blk):
                """One resnet block; NET lives in the PSUM accumulator NB,
                per-block biases are cumulative consts applied at the relu."""
                CFX = stu["CFX"]
                NB = stu["NB"]
                R0 = pl.tile([128, 512], F16, tag="r0", name="R0")
                nc.scalar.activation(out=R0[:], in_=NB[:], func=AFT.Relu,
                                     bias=bias(_CN_CB + blk))
                ph = ps.tile([128, 512], F32, tag="pmm", name="ph")
                nc.tensor.matmul(out=ph[:],
                                 lhsT=wbd[:, 256 * blk:256 * blk + 128],
                                 rhs=R0[:], start=True, stop=True)
                R1 = pl.tile([128, 512], F16, tag="r1", name="R1")
                nc.scalar.activation(out=R1[:], in_=ph[:], func=AFT.Relu,
                                     bias=bias(_CN_B0 + blk))
                nc.tensor.matmul(
                    out=NB[:],
                    lhsT=wbd[:, 256 * blk + 128:256 * blk + 256],
                    rhs=R1[:], start=False, stop=(blk == 4),
                    skip_group_check=True)
                if blk < 4:
                    nc.tensor.matmul(out=NB[:], lhsT=ident16[:], rhs=CFX[:],
                                     start=False, stop=False,
                                     skip_group_check=True)

            def emit_out(b, th, st, stu):
                NB = stu["NB"]
                RO = pl.tile([128, 512], F16, tag="r0", name="RO")
                nc.scalar.activation(out=RO[:], in_=NB[:], func=AFT.Relu,
                                     bias=bias(_CN_CB + 5))
                pb2 = pout.tile([128, 512], F32, tag="po", name="po")
                po = pb2[0:4, 0:512]
                nc.tensor.matmul(out=po, lhsT=fob[:], rhs=RO[:],
                                 start=True, stop=True)
                OSB = pl.tile([4, 512], F32, tag="osb", name="OSB")
                nc.scalar.activation(out=OSB[:], in_=po, func=AFT.Identity,
                                     bias=cst[0:4, _CN_FOB:_CN_FOB + 1])
                pb3 = pout.tile([128, 512], F32, tag="po", name="pt2")
                pt2 = pb3[:, 0:16]
                for kk in range(4):
                    nc.tensor.transpose(out=pb3[:, 4 * kk:4 * kk + 4],
                                        in_=OSB[:, 128 * kk:128 * kk + 128],
                                        identity=ident[0:4, 0:4])
                if th == 0:
                    OUTSB = pl.tile([128, 32], F32, tag="outsb", name="OUTSB")
                    st["OUTSB"] = OUTSB
                OUTSB = st["OUTSB"]
                nc.scalar.activation(out=OUTSB[:, 16 * th:16 * th + 16],
                                     in_=pt2, func=AFT.Copy)
                if th == 1:
                    nc.sync.dma_start(
                        out=o_d.ap()[b].rearrange("(q a) -> q a", a=32),
                        in_=OUTSB[:])

            def stageBpair(ua, ub, stb, stu):
                """Two units' MLPs in lockstep so neither chain clogs the
                per-engine wait queues."""
                for blk in range(5):
                    emit_block(stu[ua[0]][ua[1]], blk)
                    emit_block(stu[ub[0]][ub[1]], blk)
                    yield
                emit_out(ua[0], ua[1], stb[ua[0]], stu[ua[0]][ua[1]])
                emit_out(ub[0], ub[1], stb[ub[0]], stu[ub[0]][ub[1]])

            def run(gen):
                if gen is not None:
                    for _ in gen:
                        pass

            def interleave(*gens):
                """Round-robin emission across generators."""
                gens = [g for g in gens if g is not None]
                while gens:
                    for g in list(gens):
                        try:
                            next(g)
                        except StopIteration:
                            gens.remove(g)

            def body():
                stb = [{} for _ in range(BPC)]       # per-batch state
                stu = [[{} for _ in range(2)] for _ in range(BPC)]

                stageF(0, stb[0])
                stageF(1, stb[1])
                interleave(stageA(0, 0, stb[0], stu[0][0]),
                           stageA(0, 1, stb[0], stu[0][1]))
                interleave(stageA(1, 0, stb[1], stu[1][0]),
                           stageA(1, 1, stb[1], stu[1][1]),
                           stageBpair((0, 0), (0, 1), stb, stu))
                run(stageBpair((1, 0), (1, 1), stb, stu))

            if iters:
                with tc.For_i(0, iters, 1) as _i:
                    body()
            elif unroll:
                for _ in range(unroll):
                    body()
            else:
                body()

    nc.compile()
    return nc


def _host_consts(p, c, C_mat, fc_p_W, fc_p_b, blocks_W0, blocks_b0,
                 blocks_W1, blocks_b1, fc_out_W, fc_out_b):
    """Per-core input maps (shared const tensors + per-core slices)."""
    p = np.asarray(p, np.float32)
    c = np.asarray(c, np.float32)
    C_mat = np.asarray(C_mat, np.float32)

    # supercell table, d-major payload: [B, L, H*W, 32d x 4corner] fp16
    pad = np.zeros((B, L, H + 1, W + 1, D), np.float32)
    pad[:, :, :H, :W] = c
    ctab = np.empty((B, L, H, W, D, 4), np.float16)
    ctab[..., 0] = pad[:, :, :H, :W]
    ctab[..., 1] = pad[:, :, :H, 1:W + 1]
    ctab[..., 2] = pad[:, :, 1:H + 1, :W]
    ctab[..., 3] = pad[:, :, 1:H + 1, 1:W + 1]
    ctab = ctab.reshape(B, L, H * W, 4 * D)

    wbd = np.zeros((128, 1280), np.float16)
    for blk in range(5):
        for g in range(4):
            wbd[32 * g:32 * g + 32, 256 * blk + 32 * g:256 * blk + 32 * g + 32] = blocks_W0[blk]
            wbd[32 * g:32 * g + 32, 256 * blk + 128 + 32 * g:256 * blk + 160 + 32 * g] = blocks_W1[blk]

    emat = np.zeros((128, 1024), np.float32)
    for hh in range(8):
        for m in range(128):
            emat[hh * 16 + (m % 16), 128 * hh + m] = 1.0

    nmat = np.zeros((96, 1024), np.float16)
    for th in range(2):
        for k in range(4):
            for s2 in range(4):
                a = 16 * th + 4 * k + s2
                for j in range(3):
                    nmat[3 * a + j,
                         128 * (4 * th + k) + 32 * s2:
                         128 * (4 * th + k) + 32 * s2 + 32] = \
                        np.asarray(fc_p_W, np.float32)[j]

    fob = np.zeros((128, 4), np.float16)
    for g in range(4):
        fob[32 * g:32 * g + 32, g] = np.asarray(fc_out_W, np.float32)[:, 0]

    in_maps = []
    for cc in range(NCORES):
        cst = np.zeros((128, _CN), np.float32)
        for b in range(BPC):
            gb = BPC * cc + b
            for l in range(L):
                for ch in range(2):
                    k0 = 24 * b + (2 * l + ch) * 3
                    cst[:, k0:k0 + 3] = C_mat[gb, l, ch, :][None, :]
                den = C_mat[gb, l, 3, 0] + np.float32(0.05)
                for ch2 in range(2):
                    cst[:, _CN_RDEN + b * 8 + 2 * l + ch2] = np.float32(1.0) / den
        cum = np.asarray(fc_p_b, np.float32).copy()
        for blk in range(5):
            cst[:, _CN_B0 + blk] = np.tile(np.asarray(blocks_b0[blk], np.float32), 4)
            cst[:, _CN_CB + blk] = np.tile(cum, 4)
            cum = cum + np.asarray(blocks_b1[blk], np.float32)
        cst[:, _CN_CB + 5] = np.tile(cum, 4)
        cst[:, _CN_FOB] = np.float32(fc_out_b[0])
        in_maps.append({
            "ct": np.ascontiguousarray(
                ctab[BPC * cc:BPC * cc + BPC].reshape(BPC * 2, 2 * H * W, 4 * D)),
            "p": np.ascontiguousarray(p[BPC * cc:BPC * cc + BPC]),
            "pq": np.ascontiguousarray(p[BPC * cc:BPC * cc + BPC] / np.float32(MAX_DIM)),
            "cst": cst,
            "wbd": wbd,
            "emat": emat,
            "nmat": nmat,
            "fob": fob,
        })
    return in_maps


def kernel(p, z, c, C_mat, fc_p_W, fc_p_b, blocks_W0, blocks_b0,
           blocks_W1, blocks_b1, fc_out_W, fc_out_b):
    if "nc" not in _cache:
        _cache["nc"] = _build_nc()
    nc = _cache["nc"]
    in_maps = _host_consts(p, c, C_mat, fc_p_W, fc_p_b, blocks_W0, blocks_b0,
                           blocks_W1, blocks_b1, fc_out_W, fc_out_b)
    res = run_bass_kernel_spmd(nc, in_maps, core_ids=list(range(NCORES)))
    out = np.empty((B, T), np.float32)
    for cc in range(NCORES):
        out[BPC * cc:BPC * cc + BPC] = res.results[cc]["o"]
    return out
